# revision 43
# baseline (speedup 1.0000x reference)
"""Trainium2 Bass kernel v3: fused single-pass fp16 viscous-RHS.

Design vs v2 (two-pass fp32):
- Single pass: fluxes (tau rows + energy flux) are built in SBUF and the
  divergence is accumulated directly into PSUM by the PE via identity
  matmuls (dz/dx terms) + Dy matmuls (dy term). No DRAM round-trip.
- fp16 everywhere on-chip (tolerance is 2e-2; fp16 lands ~1e-3), which
  doubles DVE throughput and halves DMA bytes.
- Host pre-stages inputs already transposed to [y, field, z, x] fp16 with
  x-halo, so every DMA is large and fully contiguous (no transpose DMAs,
  ~6 DMA instructions per chunk instead of ~100).
- mu = MU_REF*(N/2)^2*T^0.7 is precomputed on host (kills ln/exp + act
  table loads). Scale folding: host stages u' = u/2, Tq = (CP/PR/4)*T,
  m2 = 2*mu; then tau_ij = m2*(du'-forms), e' = m2*dTq + sum tau*u' = e/2
  and the host doubles the energy output.
- y=192 rows are folded as [0:128) main tiles + 64-row tail folded into
  128 partitions as two z-half-ranges (2-plane overlap so z-shifts stay
  uniform per half).
"""

import sys

sys.path.insert(0, "/opt/trn_rl_repo")

import numpy as np

N = 192
NCORES = 8
NZ = N // NCORES          # 24 planes per core
ZC = 6                    # chunk size (output planes per chunk)
HC = ZC // 2
W = 194                   # flux x-width  (x in [-1, 193))
WI = 196                  # input x-width (x in [-2, 194))

MU_REF = 1.8e-5
PR = 0.72
CP = 1005.0
CPR = CP / PR
MUSCALE = MU_REF * (N / 2.0) ** 2   # both divergence scales folded into mu


def build_program(nz=NZ, num_devices=NCORES):
    import concourse.bacc as bacc
    import concourse.mybir as mybir
    from concourse.tile import TileContext

    f16 = mybir.dt.float16
    f32 = mybir.dt.float32
    assert nz % ZC == 0
    nchunks = nz // ZC
    nzp = nz + 4

    nc = bacc.Bacc("TRN2", target_bir_lowering=False, debug=False,
                   num_devices=num_devices)

    amain = nc.dram_tensor("amain", [128, 5, nzp, WI], f16,
                           kind="ExternalInput")
    atail = nc.dram_tensor("atail", [64, 5, nzp, WI], f16,
                           kind="ExternalInput")
    wts = nc.dram_tensor("wts", [192, 448], f16, kind="ExternalInput")
    omain = nc.dram_tensor("omain", [128, 4, nz, N], f16,
                           kind="ExternalOutput")
    otail = nc.dram_tensor("otail", [64, 4, nz, N], f16,
                           kind="ExternalOutput")

    E = ZC + 2        # extended plane count (main)
    ET = HC + 2       # extended planes per tail half (with overlap)
    IT = HC + 4       # input planes per tail half

    with TileContext(nc) as tc:
        with (
            tc.tile_pool(name="wpool", bufs=1) as wpool,
            tc.tile_pool(name="io", bufs=2) as io,
            tc.tile_pool(name="work", bufs=1) as wk,
            tc.tile_pool(name="psA", bufs=2, space="PSUM") as psA,
            tc.tile_pool(name="psC", bufs=4, space="PSUM") as psC,
        ):
            wap = wts.ap()
            wk0m0 = wpool.tile([128, 128], f16, tag="wk0m0")
            nc.sync.dma_start(out=wk0m0[:, :], in_=wap[0:128, 0:128])
            wk0m1 = wpool.tile([128, 64], f16, tag="wk0m1")
            nc.sync.dma_start(out=wk0m1[:, :], in_=wap[0:128, 128:192])
            wk1m0b0 = wpool.tile([64, 128], f16, tag="wk1m0b0")
            nc.sync.dma_start(out=wk1m0b0[:, :], in_=wap[128:192, 0:128])
            wk1m0b64 = wpool.tile([128, 128], f16, tag="wk1m0b64")
            nc.sync.dma_start(out=wk1m0b64[64:128, :], in_=wap[128:192, 0:128])
            wk1m1b0 = wpool.tile([64, 64], f16, tag="wk1m1b0")
            nc.sync.dma_start(out=wk1m1b0[:, :], in_=wap[128:192, 128:192])
            wk1m1b64 = wpool.tile([128, 64], f16, tag="wk1m1b64")
            nc.sync.dma_start(out=wk1m1b64[64:128, :],
                              in_=wap[128:192, 128:192])
            wi = wpool.tile([128, 128], f16, tag="wi")
            nc.sync.dma_start(out=wi[:, :], in_=wap[0:128, 192:320])
            wim = wpool.tile([128, 128], f16, tag="wim")
            nc.sync.dma_start(out=wim[:, :], in_=wap[0:128, 320:448])

            TT = nc.vector
            TP = nc.gpsimd

            def stage_load(c):
                z0 = c * ZC
                inm_t = io.tile([128, 5 * (E + 2) * WI], f16, tag="inm")
                inm = inm_t.rearrange("p (f z x) -> p f z x", f=5, z=E + 2)
                nc.sync.dma_start(out=inm[:, :, :, :],
                                  in_=amain.ap()[:, :, z0:z0 + E + 2, :])
                int_t = io.tile([128, 5 * IT * WI], f16, tag="int")
                inT = int_t.rearrange("p (f z x) -> p f z x", f=5, z=IT)
                nc.sync.dma_start(out=inT[0:64, :, :, :],
                                  in_=atail.ap()[:, :, z0:z0 + IT, :])
                nc.sync.dma_start(out=inT[64:128, :, :, :],
                                  in_=atail.ap()[:, :, z0 + HC:z0 + HC + IT,
                                                 :])
                s = {"z0": z0, "inm": inm, "inT": inT}
                return s

            def stage_A(s):
                inm, inT = s["inm"], s["inT"]
                dm_t = wk.tile([128, 12 * E * W], f16, tag="dm")
                dm = dm_t.rearrange("p (s z x) -> p s z x", s=12, z=E)
                dt_t = wk.tile([128, 12 * ET * W], f16, tag="dt")
                dt = dt_t.rearrange("p (s z x) -> p s z x", s=12, z=ET)
                s["dm"], s["dt"] = dm, dt
                # z/x derivatives; D slots: 0..3 dz, 4..7 dy, 8..11 dx
                TT.tensor_sub(dm[:, 0:4, :, :],
                              inm[:, 0:4, 2:2 + E, 1:1 + W],
                              inm[:, 0:4, 0:E, 1:1 + W])
                TT.tensor_sub(dt[:, 0:4, :, :],
                              inT[:, 0:4, 2:2 + ET, 1:1 + W],
                              inT[:, 0:4, 0:ET, 1:1 + W])
                TT.tensor_sub(dm[:, 8:12, :, :],
                              inm[:, 0:4, 1:1 + E, 2:2 + W],
                              inm[:, 0:4, 1:1 + E, 0:W])
                TT.tensor_sub(dt[:, 8:12, :, :],
                              inT[:, 0:4, 1:1 + ET, 2:2 + W],
                              inT[:, 0:4, 1:1 + ET, 0:W])

                # y derivatives (PE), drains into D slots 4..7
                def a_drain(sel, dst, pa):
                    src = pa.rearrange("p (a x) -> p a x", a=2)[:, :, 0:388] \
                        .rearrange("p a (f x) -> p a f x", f=2)
                    dstv = dst.rearrange("p (a f) x -> p a f x", a=2)
                    if sel:
                        TT.tensor_copy(out=dstv, in_=src)
                    else:
                        nc.scalar.copy(dstv, src)

                for lp in range(E):
                    pa = psA.tile([128, 1024], f32, tag="pa")
                    if lp < HC + 2:
                        kth, klt = 0, lp + 1
                        wkt = wk1m0b0[:, :]
                    else:
                        kth, klt = 64, lp - HC + 1
                        wkt = wk1m0b64[64:128, :]
                    for g in range(2):
                        pv = pa[:, 512 * g:512 * g + 388].rearrange(
                            "p (f x) -> p f x", f=2)
                        nc.tensor.matmul(pv[:, :, :], wk0m0[:, :],
                                         inm[:, 2 * g:2 * g + 2, lp + 1,
                                             1:1 + W],
                                         start=True, stop=False,
                                         skip_group_check=True)
                        nc.tensor.matmul(pv[:, :, :], wkt,
                                         inT[kth:kth + 64, 2 * g:2 * g + 2,
                                             klt, 1:1 + W],
                                         start=False, stop=True,
                                         skip_group_check=True)
                    a_drain(0, dm[:, 4:8, lp, :], pa)
                for lq in range(ET):
                    pa = psA.tile([128, 1024], f32, tag="pa")
                    for g in range(2):
                        for h, base in ((0, 0), (1, 64)):
                            lmain = lq + 1 + h * HC
                            pv = pa[base:base + 64,
                                    512 * g:512 * g + 388].rearrange(
                                "p (f x) -> p f x", f=2)
                            wkt = wk1m1b0[:, :] if h == 0 \
                                else wk1m1b64[64:128, :]
                            nc.tensor.matmul(pv[:, :, :], wk0m1[:, :],
                                             inm[:, 2 * g:2 * g + 2, lmain,
                                                 1:1 + W],
                                             start=True, stop=False,
                                             skip_group_check=True)
                            nc.tensor.matmul(pv[:, :, :], wkt,
                                             inT[base:base + 64,
                                                 2 * g:2 * g + 2, lq + 1,
                                                 1:1 + W],
                                             start=False, stop=True,
                                             skip_group_check=True)
                    a_drain(0, dt[:, 4:8, lq, :], pa)

            def stage_B1(s):
                # FL slots: [t00,t10,t20,ez|t01,t11,t21,ey|t02,t12,t22,ex]
                fm_t = wk.tile([128, 12 * E * W], f16, tag="fm")
                fm = fm_t.rearrange("p (s z x) -> p s z x", s=12, z=E)
                ft_t = wk.tile([128, 12 * ET * W], f16, tag="ft")
                ft = ft_t.rearrange("p (s z x) -> p s z x", s=12, z=ET)
                dvm_t = wk.tile([128, E * W], f16, tag="dv")
                dvm = dvm_t.rearrange("p (z x) -> p z x", z=E)
                dvt_t = wk.tile([128, ET * W], f16, tag="dvt")
                dvt = dvt_t.rearrange("p (z x) -> p z x", z=ET)
                s["fm"], s["ft"] = fm, ft
                for (d, f, dv, inp, ne) in (
                        (s["dm"], fm, dvm, s["inm"], E),
                        (s["dt"], ft, dvt, s["inT"], ET)):
                    m2 = inp[:, 4, 1:1 + ne, 1:1 + W]
                    m2b3 = m2.unsqueeze(1).broadcast_to((128, 3, ne, W))
                    # diag derivative slots doubled (u was staged halved;
                    # diag needs 2A' - (2/3)divu' = A'' - divu''/3)
                    TT.tensor_scalar_mul(d[:, 0:11:5, :, :],
                                         d[:, 0:11:5, :, :], 2.0)
                    TP.tensor_add(dv[:, :, :], d[:, 0, :, :], d[:, 5, :, :])
                    TP.tensor_add(dv[:, :, :], dv[:, :, :], d[:, 10, :, :])
                    TT.tensor_scalar_mul(dv[:, :, :], dv[:, :, :], 1.0 / 3.0)
                    dvb = dv.unsqueeze(1).broadcast_to((128, 3, ne, W))
                    TT.tensor_sub(f[:, 0:11:5, :, :], d[:, 0:11:5, :, :],
                                  dvb)
                    # off-diag h: v->t01(4), w->t20(2), t->t21(6)
                    TP.tensor_add(f[:, 4, :, :], d[:, 1, :, :], d[:, 4, :, :])
                    TP.tensor_add(f[:, 2, :, :], d[:, 2, :, :], d[:, 8, :, :])
                    TP.tensor_add(f[:, 6, :, :], d[:, 6, :, :], d[:, 9, :, :])
                    # tau muls: diag (0,5,10); off-diag (2,4,6); e-seeds
                    TT.tensor_mul(f[:, 0:11:5, :, :], f[:, 0:11:5, :, :],
                                  m2b3)
                    TP.tensor_mul(f[:, 2:7:2, :, :], f[:, 2:7:2, :, :], m2b3)
                    TT.tensor_mul(f[:, 3:12:4, :, :], d[:, 3:12:4, :, :],
                                  m2b3)
                    # symmetric copies t10<-t01, (t02,t12)<-(t20,t21)
                    TP.tensor_copy(out=f[:, 1, :, :], in_=f[:, 4, :, :])
                    TP.tensor_copy(out=f[:, 8:10, :, :],
                                   in_=f[:, 2:7:4, :, :])

            def stage_B2(s):
                # e_j += sum_i tau_ij * u'_i; row-1 on Pool (parallel with
                # the DVE chain), rows 0/2 on DVE sharing dead slots 8:11
                for (d, f, inp, ne) in (
                        (s["dm"], s["fm"], s["inm"], E),
                        (s["dt"], s["ft"], s["inT"], ET)):
                    p1 = d[:, 0:3, :, :]
                    ub = [inp[:, i, 1:1 + ne, 1:1 + W].unsqueeze(1)
                          .broadcast_to((128, 3, ne, W)) for i in range(3)]
                    TP.tensor_mul(p1[:, :, :, :], f[:, 1:12:4, :, :], ub[1])
                    TT.tensor_mul(d[:, 4:7, :, :], f[:, 2:12:4, :, :],
                                  ub[2])
                    TT.tensor_add(f[:, 3:12:4, :, :],
                                  f[:, 3:12:4, :, :], d[:, 4:7, :, :])
                    TT.tensor_mul(d[:, 8:11, :, :], f[:, 0:12:4, :, :],
                                  ub[0])
                    TT.tensor_add(f[:, 3:12:4, :, :],
                                  f[:, 3:12:4, :, :], d[:, 8:11, :, :])
                    TP.tensor_add(f[:, 3:12:4, :, :], f[:, 3:12:4, :, :],
                                  p1[:, :, :, :])

            def stage_C(s, g):
                fm, ft = s["fm"], s["ft"]
                if g == 0:
                    om_t = wk.tile([128, 4 * ZC * N], f16, tag="om")
                    s["om"] = om_t.rearrange("p (f z x) -> p f z x",
                                             f=4, z=ZC)
                    ot_t = wk.tile([128, 4 * HC * N], f16, tag="ot")
                    s["ot"] = ot_t.rearrange("p (f z x) -> p f z x",
                                             f=4, z=HC)
                om, ot = s["om"], s["ot"]
                q = 2 * g
                for k in range(ZC):
                    le = k + 1
                    pc = psC.tile([128, 512], f32, tag="pc")
                    pv = pc[:, 0:384].rearrange("p (f x) -> p f x", f=2)
                    if k < HC:
                        kth, klt = 0, k + 1
                        wkt = wk1m0b0[:, :]
                    else:
                        kth, klt = 64, k - HC + 1
                        wkt = wk1m0b64[64:128, :]
                    nc.tensor.matmul(pv[:, :, :], wi[:, :],
                                     fm[:, q:q + 2, le + 1, 1:193],
                                     start=True, stop=False,
                                     skip_group_check=True)
                    nc.tensor.matmul(pv[:, :, :], wim[:, :],
                                     fm[:, q:q + 2, le - 1, 1:193],
                                     start=False, stop=False,
                                     skip_group_check=True)
                    nc.tensor.matmul(pv[:, :, :], wi[:, :],
                                     fm[:, 8 + q:10 + q, le, 2:194],
                                     start=False, stop=False,
                                     skip_group_check=True)
                    nc.tensor.matmul(pv[:, :, :], wim[:, :],
                                     fm[:, 8 + q:10 + q, le, 0:192],
                                     start=False, stop=False,
                                     skip_group_check=True)
                    nc.tensor.matmul(pv[:, :, :], wk0m0[:, :],
                                     fm[:, 4 + q:6 + q, le, 1:193],
                                     start=False, stop=False,
                                     skip_group_check=True)
                    nc.tensor.matmul(pv[:, :, :], wkt,
                                     ft[kth:kth + 64, 4 + q:6 + q, klt,
                                        1:193],
                                     start=False, stop=True,
                                     skip_group_check=True)
                    nc.scalar.copy(om[:, q:q + 2, k, :], pv[:, :, :])
                for k in range(HC):
                    le = k + 1
                    pc = psC.tile([128, 512], f32, tag="pc")
                    pv = pc[:, 0:384].rearrange("p (f x) -> p f x", f=2)
                    nc.tensor.matmul(pv[:, :, :], wi[:, :],
                                     ft[:, q:q + 2, le + 1, 1:193],
                                     start=True, stop=False,
                                     skip_group_check=True)
                    nc.tensor.matmul(pv[:, :, :], wim[:, :],
                                     ft[:, q:q + 2, le - 1, 1:193],
                                     start=False, stop=False,
                                     skip_group_check=True)
                    nc.tensor.matmul(pv[:, :, :], wi[:, :],
                                     ft[:, 8 + q:10 + q, le, 2:194],
                                     start=False, stop=False,
                                     skip_group_check=True)
                    nc.tensor.matmul(pv[:, :, :], wim[:, :],
                                     ft[:, 8 + q:10 + q, le, 0:192],
                                     start=False, stop=False,
                                     skip_group_check=True)
                    for h, base in ((0, 0), (1, 64)):
                        pvh = pc[base:base + 64, 0:384].rearrange(
                            "p (f x) -> p f x", f=2)
                        wkt = wk1m1b0[:, :] if h == 0 \
                            else wk1m1b64[64:128, :]
                        nc.tensor.matmul(pvh[:, :, :], wk0m1[:, :],
                                         fm[:, 4 + q:6 + q,
                                            k + 1 + h * HC, 1:193],
                                         start=False, stop=False,
                                         skip_group_check=True)
                        nc.tensor.matmul(pvh[:, :, :], wkt,
                                         ft[base:base + 64, 4 + q:6 + q,
                                            le, 1:193],
                                         start=False, stop=(h == 1),
                                         skip_group_check=True)
                    nc.scalar.copy(ot[:, q:q + 2, k, :], pv[:, :, :])

            def stage_store(s):
                z0 = s["z0"]
                nc.sync.dma_start(out=omain.ap()[:, :, z0:z0 + ZC, :],
                                  in_=s["om"][:, :, :, :])
                nc.sync.dma_start(out=otail.ap()[:, :, z0:z0 + HC, :],
                                  in_=s["ot"][0:64, :, :, :])
                nc.sync.dma_start(out=otail.ap()[:, :, z0 + HC:z0 + ZC, :],
                                  in_=s["ot"][64:128, :, :, :])

            # software-pipelined emission: A(c+1) is emitted between B2(c)
            # and C-g1(c) so PE/ACT fill B2's shadow and B1(c+1) can start
            # during C-g1(c)
            st = stage_load(0)
            stage_A(st)
            for c in range(nchunks):
                nxt = stage_load(c + 1) if c + 1 < nchunks else None
                stage_B1(st)
                stage_C(st, 0)
                stage_B2(st)
                if nxt is not None:
                    stage_A(nxt)
                stage_C(st, 1)
                stage_store(st)
                st = nxt

    nc.compile()
    return nc


_NC_CACHE = None


def _get_nc():
    global _NC_CACHE
    if _NC_CACHE is None:
        _NC_CACHE = build_program()
    return _NC_CACHE


def make_wts() -> np.ndarray:
    dy = np.zeros((N, N), dtype=np.float32)
    for m in range(N):
        dy[m, (m + 1) % N] = 1.0
        dy[m, (m - 1) % N] = -1.0
    dyt = dy.T
    eye = np.eye(128, dtype=np.float32)
    w = np.zeros((192, 448), dtype=np.float32)
    w[:, 0:192] = dyt
    w[0:128, 192:320] = eye
    w[0:128, 320:448] = -eye
    return w.astype(np.float16)


def stage_fields(u, T):
    """Full-grid staged fields [5, N, N, WI] fp16 (x-halo'd, scaled)."""
    mu2 = (2.0 * MUSCALE) * np.power(T, 0.7, dtype=np.float32)
    tq = (CPR / 4.0) * T
    f = np.empty((5, N, N, N), dtype=np.float32)
    f[0:3] = 0.5 * u
    f[3] = tq
    f[4] = mu2
    fx = np.concatenate([f[..., -2:], f, f[..., :2]], axis=-1)
    return fx.astype(np.float16)


def shard_inputs(u, T, nz=NZ, ncores=NCORES):
    fx = stage_fields(u, T)          # [5, N, N, WI]
    wts = make_wts()
    in_maps = []
    for k in range(ncores):
        idx = np.arange(nz * k - 2, nz * k + nz + 2) % N
        blk = fx[:, idx, :, :]                     # [5, nz+4, N, WI]
        blk = np.ascontiguousarray(blk.transpose(2, 0, 1, 3))  # [y,5,z,x]
        in_maps.append({
            "amain": blk[0:128],
            "atail": np.ascontiguousarray(blk[128:192]),
            "wts": wts,
        })
    return in_maps


def kernel(u: np.ndarray, T: np.ndarray) -> np.ndarray:
    from concourse.bass_utils import run_bass_kernel_spmd

    u = np.asarray(u, dtype=np.float32)
    T = np.asarray(T, dtype=np.float32)
    nc = _get_nc()
    in_maps = shard_inputs(u, T)
    res = run_bass_kernel_spmd(nc, in_maps, list(range(NCORES)))

    out = np.zeros((5, N, N, N), dtype=np.float32)
    for k in range(NCORES):
        r = res.results[k]
        o = np.concatenate([np.asarray(r["omain"], dtype=np.float32),
                            np.asarray(r["otail"], dtype=np.float32)],
                           axis=0)                  # [192y, 4, nz, x]
        o = o.transpose(1, 2, 0, 3)                 # [4, nz, y, x]
        out[1:5, NZ * k:NZ * k + NZ] = o
    out[4] *= 2.0
    return out


# revision 45
# speedup vs baseline: 1.0292x; 1.0292x over previous
"""Trainium2 Bass kernel v3: fused single-pass fp16 viscous-RHS.

Design vs v2 (two-pass fp32):
- Single pass: fluxes (tau rows + energy flux) are built in SBUF and the
  divergence is accumulated directly into PSUM by the PE via identity
  matmuls (dz/dx terms) + Dy matmuls (dy term). No DRAM round-trip.
- fp16 everywhere on-chip (tolerance is 2e-2; fp16 lands ~1e-3), which
  doubles DVE throughput and halves DMA bytes.
- Host pre-stages inputs already transposed to [y, field, z, x] fp16 with
  x-halo, so every DMA is large and fully contiguous (no transpose DMAs,
  ~6 DMA instructions per chunk instead of ~100).
- mu = MU_REF*(N/2)^2*T^0.7 is precomputed on host (kills ln/exp + act
  table loads). Scale folding: host stages u' = u/2, Tq = (CP/PR/4)*T,
  m2 = 2*mu; then tau_ij = m2*(du'-forms), e' = m2*dTq + sum tau*u' = e/2
  and the host doubles the energy output.
- y=192 rows are folded as [0:128) main tiles + 64-row tail folded into
  128 partitions as two z-half-ranges (2-plane overlap so z-shifts stay
  uniform per half).
"""

import sys

sys.path.insert(0, "/opt/trn_rl_repo")

import numpy as np

N = 192
NCORES = 8
NZ = N // NCORES          # 24 planes per core
ZC = 6                    # chunk size (output planes per chunk)
HC = ZC // 2
W = 194                   # flux x-width  (x in [-1, 193))
WI = 196                  # input x-width (x in [-2, 194))

MU_REF = 1.8e-5
PR = 0.72
CP = 1005.0
CPR = CP / PR
MUSCALE = MU_REF * (N / 2.0) ** 2   # both divergence scales folded into mu


def build_program(nz=NZ, num_devices=NCORES):
    import concourse.bacc as bacc
    import concourse.mybir as mybir
    from concourse.tile import TileContext

    f16 = mybir.dt.float16
    f32 = mybir.dt.float32
    assert nz % ZC == 0
    nchunks = nz // ZC
    nzp = nz + 4

    nc = bacc.Bacc("TRN2", target_bir_lowering=False, debug=False,
                   num_devices=num_devices)

    amain = nc.dram_tensor("amain", [128, 5, nzp, WI], f16,
                           kind="ExternalInput")
    atail = nc.dram_tensor("atail", [64, 5, nzp, WI], f16,
                           kind="ExternalInput")
    wts = nc.dram_tensor("wts", [192, 448], f16, kind="ExternalInput")
    omain = nc.dram_tensor("omain", [128, 4, nz, N], f16,
                           kind="ExternalOutput")
    otail = nc.dram_tensor("otail", [64, 4, nz, N], f16,
                           kind="ExternalOutput")

    E = ZC + 2        # extended plane count (main)
    ET = HC + 2       # extended planes per tail half (with overlap)
    IT = HC + 4       # input planes per tail half

    with TileContext(nc) as tc:
        with (
            tc.tile_pool(name="wpool", bufs=1) as wpool,
            tc.tile_pool(name="io", bufs=2) as io,
            tc.tile_pool(name="work", bufs=1) as wk,
            tc.tile_pool(name="psA", bufs=2, space="PSUM") as psA,
            tc.tile_pool(name="psC", bufs=4, space="PSUM") as psC,
        ):
            wap = wts.ap()
            wk0m0 = wpool.tile([128, 128], f16, tag="wk0m0")
            nc.sync.dma_start(out=wk0m0[:, :], in_=wap[0:128, 0:128])
            wk0m1 = wpool.tile([128, 64], f16, tag="wk0m1")
            nc.sync.dma_start(out=wk0m1[:, :], in_=wap[0:128, 128:192])
            wk1m0b0 = wpool.tile([64, 128], f16, tag="wk1m0b0")
            nc.sync.dma_start(out=wk1m0b0[:, :], in_=wap[128:192, 0:128])
            wk1m0b64 = wpool.tile([128, 128], f16, tag="wk1m0b64")
            nc.sync.dma_start(out=wk1m0b64[64:128, :], in_=wap[128:192, 0:128])
            wk1m1b0 = wpool.tile([64, 64], f16, tag="wk1m1b0")
            nc.sync.dma_start(out=wk1m1b0[:, :], in_=wap[128:192, 128:192])
            wk1m1b64 = wpool.tile([128, 64], f16, tag="wk1m1b64")
            nc.sync.dma_start(out=wk1m1b64[64:128, :],
                              in_=wap[128:192, 128:192])
            wi = wpool.tile([128, 128], f16, tag="wi")
            nc.sync.dma_start(out=wi[:, :], in_=wap[0:128, 192:320])
            wim = wpool.tile([128, 128], f16, tag="wim")
            nc.sync.dma_start(out=wim[:, :], in_=wap[0:128, 320:448])

            TT = nc.vector
            TP = nc.gpsimd

            def stage_load(c):
                z0 = c * ZC
                inm_t = io.tile([128, 5 * (E + 2) * WI], f16, tag="inm")
                inm = inm_t.rearrange("p (f z x) -> p f z x", f=5, z=E + 2)
                nc.sync.dma_start(out=inm[:, :, :, :],
                                  in_=amain.ap()[:, :, z0:z0 + E + 2, :])
                int_t = io.tile([128, 5 * IT * WI], f16, tag="int")
                inT = int_t.rearrange("p (f z x) -> p f z x", f=5, z=IT)
                nc.sync.dma_start(out=inT[0:64, :, :, :],
                                  in_=atail.ap()[:, :, z0:z0 + IT, :])
                nc.sync.dma_start(out=inT[64:128, :, :, :],
                                  in_=atail.ap()[:, :, z0 + HC:z0 + HC + IT,
                                                 :])
                s = {"z0": z0, "inm": inm, "inT": inT}
                return s

            def stage_A(s):
                inm, inT = s["inm"], s["inT"]
                dm_t = wk.tile([128, 12 * E * W], f16, tag="dm")
                dm = dm_t.rearrange("p (s z x) -> p s z x", s=12, z=E)
                dt_t = wk.tile([128, 12 * ET * W], f16, tag="dt")
                dt = dt_t.rearrange("p (s z x) -> p s z x", s=12, z=ET)
                s["dm"], s["dt"] = dm, dt
                # z/x derivatives; D slots: 0..3 dz, 4..7 dy, 8..11 dx
                TT.tensor_sub(dm[:, 0:4, :, :],
                              inm[:, 0:4, 2:2 + E, 1:1 + W],
                              inm[:, 0:4, 0:E, 1:1 + W])
                TT.tensor_sub(dt[:, 0:4, :, :],
                              inT[:, 0:4, 2:2 + ET, 1:1 + W],
                              inT[:, 0:4, 0:ET, 1:1 + W])
                TT.tensor_sub(dm[:, 8:12, :, :],
                              inm[:, 0:4, 1:1 + E, 2:2 + W],
                              inm[:, 0:4, 1:1 + E, 0:W])
                TT.tensor_sub(dt[:, 8:12, :, :],
                              inT[:, 0:4, 1:1 + ET, 2:2 + W],
                              inT[:, 0:4, 1:1 + ET, 0:W])

                # y derivatives (PE), drains into D slots 4..7
                def a_drain(sel, dst, pa):
                    src = pa.rearrange("p (a x) -> p a x", a=2)[:, :, 0:388] \
                        .rearrange("p a (f x) -> p a f x", f=2)
                    dstv = dst.rearrange("p (a f) x -> p a f x", a=2)
                    if sel:
                        TT.tensor_copy(out=dstv, in_=src)
                    else:
                        nc.scalar.copy(dstv, src)

                for lp in range(E):
                    pa = psA.tile([128, 1024], f32, tag="pa")
                    if lp < HC + 2:
                        kth, klt = 0, lp + 1
                        wkt = wk1m0b0[:, :]
                    else:
                        kth, klt = 64, lp - HC + 1
                        wkt = wk1m0b64[64:128, :]
                    for g in range(2):
                        pv = pa[:, 512 * g:512 * g + 388].rearrange(
                            "p (f x) -> p f x", f=2)
                        nc.tensor.matmul(pv[:, :, :], wk0m0[:, :],
                                         inm[:, 2 * g:2 * g + 2, lp + 1,
                                             1:1 + W],
                                         start=True, stop=False,
                                         skip_group_check=True)
                        nc.tensor.matmul(pv[:, :, :], wkt,
                                         inT[kth:kth + 64, 2 * g:2 * g + 2,
                                             klt, 1:1 + W],
                                         start=False, stop=True,
                                         skip_group_check=True)
                    a_drain(0, dm[:, 4:8, lp, :], pa)
                for lq in range(ET):
                    pa = psA.tile([128, 1024], f32, tag="pa")
                    for g in range(2):
                        for h, base in ((0, 0), (1, 64)):
                            lmain = lq + 1 + h * HC
                            pv = pa[base:base + 64,
                                    512 * g:512 * g + 388].rearrange(
                                "p (f x) -> p f x", f=2)
                            wkt = wk1m1b0[:, :] if h == 0 \
                                else wk1m1b64[64:128, :]
                            nc.tensor.matmul(pv[:, :, :], wk0m1[:, :],
                                             inm[:, 2 * g:2 * g + 2, lmain,
                                                 1:1 + W],
                                             start=True, stop=False,
                                             skip_group_check=True)
                            nc.tensor.matmul(pv[:, :, :], wkt,
                                             inT[base:base + 64,
                                                 2 * g:2 * g + 2, lq + 1,
                                                 1:1 + W],
                                             start=False, stop=True,
                                             skip_group_check=True)
                    a_drain(0, dt[:, 4:8, lq, :], pa)

            def stage_B1(s):
                # FL slots: [t00,t10,t20,ez|t01,t11,t21,ey|t02,t12,t22,ex]
                fm_t = wk.tile([128, 12 * E * W], f16, tag="fm")
                fm = fm_t.rearrange("p (s z x) -> p s z x", s=12, z=E)
                ft_t = wk.tile([128, 12 * ET * W], f16, tag="ft")
                ft = ft_t.rearrange("p (s z x) -> p s z x", s=12, z=ET)
                dvm_t = wk.tile([128, E * W], f16, tag="dv")
                dvm = dvm_t.rearrange("p (z x) -> p z x", z=E)
                dvt_t = wk.tile([128, ET * W], f16, tag="dvt")
                dvt = dvt_t.rearrange("p (z x) -> p z x", z=ET)
                s["fm"], s["ft"] = fm, ft
                s["dvm"], s["dvt"] = dvm, dvt
                for (d, f, dv, inp, ne) in (
                        (s["dm"], fm, dvm, s["inm"], E),
                        (s["dt"], ft, dvt, s["inT"], ET)):
                    m2 = inp[:, 4, 1:1 + ne, 1:1 + W]
                    m2b3 = m2.unsqueeze(1).broadcast_to((128, 3, ne, W))
                    # diag derivative slots doubled (u was staged halved;
                    # diag needs 2A' - (2/3)divu' = A'' - divu''/3)
                    TT.tensor_scalar_mul(d[:, 0:11:5, :, :],
                                         d[:, 0:11:5, :, :], 2.0)
                    TP.tensor_add(dv[:, :, :], d[:, 0, :, :], d[:, 5, :, :])
                    TP.tensor_add(dv[:, :, :], dv[:, :, :], d[:, 10, :, :])
                    TT.tensor_scalar_mul(dv[:, :, :], dv[:, :, :], 1.0 / 3.0)
                    # s00 needs full E (feeds Fz); s11/s22 interior only
                    lo, hi = 1, ne - 1
                    dvb2 = dv[:, lo:hi, :].unsqueeze(1).broadcast_to(
                        (128, 2, hi - lo, W))
                    TT.tensor_sub(f[:, 0, :, :], d[:, 0, :, :], dv[:, :, :])
                    TT.tensor_sub(f[:, 5:11:5, lo:hi, :],
                                  d[:, 5:11:5, lo:hi, :], dvb2)
                    # off-diag h: v->t01(4), w->t20(2), t->t21(6)
                    TP.tensor_add(f[:, 4, :, :], d[:, 1, :, :], d[:, 4, :, :])
                    TP.tensor_add(f[:, 2, :, :], d[:, 2, :, :], d[:, 8, :, :])
                    TP.tensor_add(f[:, 6, :, :], d[:, 6, :, :], d[:, 9, :, :])
                    # tau muls: diag (0,5,10); off-diag (2,4,6); e-seeds
                    TT.tensor_mul(f[:, 0, :, :], f[:, 0, :, :], m2)
                    TT.tensor_mul(f[:, 5:11:5, lo:hi, :],
                                  f[:, 5:11:5, lo:hi, :],
                                  m2[:, lo:hi, :].unsqueeze(1).broadcast_to(
                                      (128, 2, hi - lo, W)))
                    TP.tensor_mul(f[:, 2:7:2, :, :], f[:, 2:7:2, :, :], m2b3)
                    TT.tensor_mul(f[:, 3:12:4, :, :], d[:, 3:12:4, :, :],
                                  m2b3)
                    # symmetric copies t10<-t01, (t02,t12)<-(t20,t21)
                    TP.tensor_copy(out=f[:, 1, :, :], in_=f[:, 4, :, :])
                    TP.tensor_copy(out=f[:, 8:10, :, :],
                                   in_=f[:, 2:7:4, :, :])

            def stage_B2(s):
                # e_j += sum_i tau_ij * u'_i; row-1 on Pool (parallel with
                # the DVE chain), rows 0/2 on DVE sharing dead slots 8:11
                for (d, f, dv, inp, ne) in (
                        (s["dm"], s["fm"], s["dvm"], s["inm"], E),
                        (s["dt"], s["ft"], s["dvt"], s["inT"], ET)):
                    # interior planes only for the full 3x3 product block
                    lo, hi = 1, ne - 1
                    p1 = d[:, 0:3, lo:hi, :]
                    nei = hi - lo
                    ub = [inp[:, i, 1 + lo:1 + hi, 1:1 + W].unsqueeze(1)
                          .broadcast_to((128, 3, nei, W)) for i in range(3)]
                    ev = f[:, 3:12:4, lo:hi, :]
                    TP.tensor_mul(p1[:, :, :, :],
                                  f[:, 1:12:4, lo:hi, :], ub[1])
                    TT.tensor_mul(d[:, 4:7, lo:hi, :],
                                  f[:, 2:12:4, lo:hi, :], ub[2])
                    TT.tensor_add(ev, ev, d[:, 4:7, lo:hi, :])
                    TT.tensor_mul(d[:, 8:11, lo:hi, :],
                                  f[:, 0:12:4, lo:hi, :], ub[0])
                    TT.tensor_add(ev, ev, d[:, 8:11, lo:hi, :])
                    TP.tensor_add(ev, ev, p1[:, :, :, :])
                    # edge planes (locals 0 and ne-1): only ez is consumed
                    # there (C's dz stream); add tau_i0*u'_i via dv scratch
                    zs = ne - 1
                    eze = f[:, 3, 0:ne:zs, :]
                    for i in range(3):
                        ue = inp[:, i, 1:ne + 1:zs, 1:1 + W]
                        TT.tensor_mul(dv[:, 0:2, :], f[:, i, 0:ne:zs, :], ue)
                        TT.tensor_add(eze, eze, dv[:, 0:2, :])

            def stage_C(s, g):
                fm, ft = s["fm"], s["ft"]
                if g == 0:
                    om_t = wk.tile([128, 4 * ZC * N], f16, tag="om")
                    s["om"] = om_t.rearrange("p (f z x) -> p f z x",
                                             f=4, z=ZC)
                    ot_t = wk.tile([128, 4 * HC * N], f16, tag="ot")
                    s["ot"] = ot_t.rearrange("p (f z x) -> p f z x",
                                             f=4, z=HC)
                om, ot = s["om"], s["ot"]
                q = 2 * g
                for k in range(ZC):
                    le = k + 1
                    pc = psC.tile([128, 512], f32, tag="pc")
                    pv = pc[:, 0:384].rearrange("p (f x) -> p f x", f=2)
                    if k < HC:
                        kth, klt = 0, k + 1
                        wkt = wk1m0b0[:, :]
                    else:
                        kth, klt = 64, k - HC + 1
                        wkt = wk1m0b64[64:128, :]
                    nc.tensor.matmul(pv[:, :, :], wi[:, :],
                                     fm[:, q:q + 2, le + 1, 1:193],
                                     start=True, stop=False,
                                     skip_group_check=True)
                    nc.tensor.matmul(pv[:, :, :], wim[:, :],
                                     fm[:, q:q + 2, le - 1, 1:193],
                                     start=False, stop=False,
                                     skip_group_check=True)
                    nc.tensor.matmul(pv[:, :, :], wi[:, :],
                                     fm[:, 8 + q:10 + q, le, 2:194],
                                     start=False, stop=False,
                                     skip_group_check=True)
                    nc.tensor.matmul(pv[:, :, :], wim[:, :],
                                     fm[:, 8 + q:10 + q, le, 0:192],
                                     start=False, stop=False,
                                     skip_group_check=True)
                    nc.tensor.matmul(pv[:, :, :], wk0m0[:, :],
                                     fm[:, 4 + q:6 + q, le, 1:193],
                                     start=False, stop=False,
                                     skip_group_check=True)
                    nc.tensor.matmul(pv[:, :, :], wkt,
                                     ft[kth:kth + 64, 4 + q:6 + q, klt,
                                        1:193],
                                     start=False, stop=True,
                                     skip_group_check=True)
                    nc.scalar.copy(om[:, q:q + 2, k, :], pv[:, :, :])
                for k in range(HC):
                    le = k + 1
                    pc = psC.tile([128, 512], f32, tag="pc")
                    pv = pc[:, 0:384].rearrange("p (f x) -> p f x", f=2)
                    nc.tensor.matmul(pv[:, :, :], wi[:, :],
                                     ft[:, q:q + 2, le + 1, 1:193],
                                     start=True, stop=False,
                                     skip_group_check=True)
                    nc.tensor.matmul(pv[:, :, :], wim[:, :],
                                     ft[:, q:q + 2, le - 1, 1:193],
                                     start=False, stop=False,
                                     skip_group_check=True)
                    nc.tensor.matmul(pv[:, :, :], wi[:, :],
                                     ft[:, 8 + q:10 + q, le, 2:194],
                                     start=False, stop=False,
                                     skip_group_check=True)
                    nc.tensor.matmul(pv[:, :, :], wim[:, :],
                                     ft[:, 8 + q:10 + q, le, 0:192],
                                     start=False, stop=False,
                                     skip_group_check=True)
                    for h, base in ((0, 0), (1, 64)):
                        pvh = pc[base:base + 64, 0:384].rearrange(
                            "p (f x) -> p f x", f=2)
                        wkt = wk1m1b0[:, :] if h == 0 \
                            else wk1m1b64[64:128, :]
                        nc.tensor.matmul(pvh[:, :, :], wk0m1[:, :],
                                         fm[:, 4 + q:6 + q,
                                            k + 1 + h * HC, 1:193],
                                         start=False, stop=False,
                                         skip_group_check=True)
                        nc.tensor.matmul(pvh[:, :, :], wkt,
                                         ft[base:base + 64, 4 + q:6 + q,
                                            le, 1:193],
                                         start=False, stop=(h == 1),
                                         skip_group_check=True)
                    nc.scalar.copy(ot[:, q:q + 2, k, :], pv[:, :, :])

            def stage_store(s):
                z0 = s["z0"]
                nc.sync.dma_start(out=omain.ap()[:, :, z0:z0 + ZC, :],
                                  in_=s["om"][:, :, :, :])
                nc.sync.dma_start(out=otail.ap()[:, :, z0:z0 + HC, :],
                                  in_=s["ot"][0:64, :, :, :])
                nc.sync.dma_start(out=otail.ap()[:, :, z0 + HC:z0 + ZC, :],
                                  in_=s["ot"][64:128, :, :, :])

            # software-pipelined emission: A(c+1) is emitted between B2(c)
            # and C-g1(c) so PE/ACT fill B2's shadow and B1(c+1) can start
            # during C-g1(c)
            st = stage_load(0)
            stage_A(st)
            for c in range(nchunks):
                nxt = stage_load(c + 1) if c + 1 < nchunks else None
                stage_B1(st)
                stage_C(st, 0)
                stage_B2(st)
                if nxt is not None:
                    stage_A(nxt)
                stage_C(st, 1)
                stage_store(st)
                st = nxt

    nc.compile()
    return nc


_NC_CACHE = None


def _get_nc():
    global _NC_CACHE
    if _NC_CACHE is None:
        _NC_CACHE = build_program()
    return _NC_CACHE


def make_wts() -> np.ndarray:
    dy = np.zeros((N, N), dtype=np.float32)
    for m in range(N):
        dy[m, (m + 1) % N] = 1.0
        dy[m, (m - 1) % N] = -1.0
    dyt = dy.T
    eye = np.eye(128, dtype=np.float32)
    w = np.zeros((192, 448), dtype=np.float32)
    w[:, 0:192] = dyt
    w[0:128, 192:320] = eye
    w[0:128, 320:448] = -eye
    return w.astype(np.float16)


def stage_fields(u, T):
    """Full-grid staged fields [5, N, N, WI] fp16 (x-halo'd, scaled)."""
    mu2 = (2.0 * MUSCALE) * np.power(T, 0.7, dtype=np.float32)
    tq = (CPR / 4.0) * T
    f = np.empty((5, N, N, N), dtype=np.float32)
    f[0:3] = 0.5 * u
    f[3] = tq
    f[4] = mu2
    fx = np.concatenate([f[..., -2:], f, f[..., :2]], axis=-1)
    return fx.astype(np.float16)


def shard_inputs(u, T, nz=NZ, ncores=NCORES):
    fx = stage_fields(u, T)          # [5, N, N, WI]
    wts = make_wts()
    in_maps = []
    for k in range(ncores):
        idx = np.arange(nz * k - 2, nz * k + nz + 2) % N
        blk = fx[:, idx, :, :]                     # [5, nz+4, N, WI]
        blk = np.ascontiguousarray(blk.transpose(2, 0, 1, 3))  # [y,5,z,x]
        in_maps.append({
            "amain": blk[0:128],
            "atail": np.ascontiguousarray(blk[128:192]),
            "wts": wts,
        })
    return in_maps


def kernel(u: np.ndarray, T: np.ndarray) -> np.ndarray:
    from concourse.bass_utils import run_bass_kernel_spmd

    u = np.asarray(u, dtype=np.float32)
    T = np.asarray(T, dtype=np.float32)
    nc = _get_nc()
    in_maps = shard_inputs(u, T)
    res = run_bass_kernel_spmd(nc, in_maps, list(range(NCORES)))

    out = np.zeros((5, N, N, N), dtype=np.float32)
    for k in range(NCORES):
        r = res.results[k]
        o = np.concatenate([np.asarray(r["omain"], dtype=np.float32),
                            np.asarray(r["otail"], dtype=np.float32)],
                           axis=0)                  # [192y, 4, nz, x]
        o = o.transpose(1, 2, 0, 3)                 # [4, nz, y, x]
        out[1:5, NZ * k:NZ * k + NZ] = o
    out[4] *= 2.0
    return out


# revision 47
# speedup vs baseline: 1.0981x; 1.0669x over previous
"""Trainium2 Bass kernel v3: fused single-pass fp16 viscous-RHS.

Design vs v2 (two-pass fp32):
- Single pass: fluxes (tau rows + energy flux) are built in SBUF and the
  divergence is accumulated directly into PSUM by the PE via identity
  matmuls (dz/dx terms) + Dy matmuls (dy term). No DRAM round-trip.
- fp16 everywhere on-chip (tolerance is 2e-2; fp16 lands ~1e-3), which
  doubles DVE throughput and halves DMA bytes.
- Host pre-stages inputs already transposed to [y, field, z, x] fp16 with
  x-halo, so every DMA is large and fully contiguous (no transpose DMAs,
  ~6 DMA instructions per chunk instead of ~100).
- mu = MU_REF*(N/2)^2*T^0.7 is precomputed on host (kills ln/exp + act
  table loads). Scale folding: host stages u' = u/2, Tq = (CP/PR/4)*T,
  m2 = 2*mu; then tau_ij = m2*(du'-forms), e' = m2*dTq + sum tau*u' = e/2
  and the host doubles the energy output.
- y=192 rows are folded as [0:128) main tiles + 64-row tail folded into
  128 partitions as two z-half-ranges (2-plane overlap so z-shifts stay
  uniform per half).
"""

import sys

sys.path.insert(0, "/opt/trn_rl_repo")

import numpy as np

N = 192
NCORES = 8
NZ = N // NCORES          # 24 planes per core
ZC = 6                    # chunk size (output planes per chunk)
HC = ZC // 2
W = 194                   # flux x-width  (x in [-1, 193))
WI = 196                  # input x-width (x in [-2, 194))

MU_REF = 1.8e-5
PR = 0.72
CP = 1005.0
CPR = CP / PR
MUSCALE = MU_REF * (N / 2.0) ** 2   # both divergence scales folded into mu


def build_program(nz=NZ, num_devices=NCORES):
    import concourse.bacc as bacc
    import concourse.mybir as mybir
    from concourse.tile import TileContext

    f16 = mybir.dt.float16
    f32 = mybir.dt.float32
    assert nz % ZC == 0
    nchunks = nz // ZC
    nzp = nz + 4

    nc = bacc.Bacc("TRN2", target_bir_lowering=False, debug=False,
                   num_devices=num_devices)

    amain = nc.dram_tensor("amain", [128, 5, nzp, WI], f16,
                           kind="ExternalInput")
    atail = nc.dram_tensor("atail", [64, 5, nzp, WI], f16,
                           kind="ExternalInput")
    wts = nc.dram_tensor("wts", [192, 448], f16, kind="ExternalInput")
    omain = nc.dram_tensor("omain", [128, 4, nz, N], f16,
                           kind="ExternalOutput")
    otail = nc.dram_tensor("otail", [64, 4, nz, N], f16,
                           kind="ExternalOutput")

    E = ZC + 2        # extended plane count (main)
    ET = HC + 2       # extended planes per tail half (with overlap)
    IT = HC + 4       # input planes per tail half

    with TileContext(nc) as tc:
        with (
            tc.tile_pool(name="wpool", bufs=1) as wpool,
            tc.tile_pool(name="io", bufs=2) as io,
            tc.tile_pool(name="work", bufs=1) as wk,
            tc.tile_pool(name="psA", bufs=2, space="PSUM") as psA,
            tc.tile_pool(name="psC", bufs=4, space="PSUM") as psC,
        ):
            wap = wts.ap()
            wk0m0 = wpool.tile([128, 128], f16, tag="wk0m0")
            nc.sync.dma_start(out=wk0m0[:, :], in_=wap[0:128, 0:128])
            wk0m1 = wpool.tile([128, 64], f16, tag="wk0m1")
            nc.sync.dma_start(out=wk0m1[:, :], in_=wap[0:128, 128:192])
            wk1m0b0 = wpool.tile([64, 128], f16, tag="wk1m0b0")
            nc.sync.dma_start(out=wk1m0b0[:, :], in_=wap[128:192, 0:128])
            wk1m0b64 = wpool.tile([128, 128], f16, tag="wk1m0b64")
            nc.sync.dma_start(out=wk1m0b64[64:128, :], in_=wap[128:192, 0:128])
            wk1m1b0 = wpool.tile([64, 64], f16, tag="wk1m1b0")
            nc.sync.dma_start(out=wk1m1b0[:, :], in_=wap[128:192, 128:192])
            wk1m1b64 = wpool.tile([128, 64], f16, tag="wk1m1b64")
            nc.sync.dma_start(out=wk1m1b64[64:128, :],
                              in_=wap[128:192, 128:192])
            wi = wpool.tile([128, 128], f16, tag="wi")
            nc.sync.dma_start(out=wi[:, :], in_=wap[0:128, 192:320])
            wim = wpool.tile([128, 128], f16, tag="wim")
            nc.sync.dma_start(out=wim[:, :], in_=wap[0:128, 320:448])

            TT = nc.vector
            TP = nc.gpsimd

            def stage_load(c):
                z0 = c * ZC
                inm_t = io.tile([128, 5 * (E + 2) * WI], f16, tag="inm")
                inm = inm_t.rearrange("p (f z x) -> p f z x", f=5, z=E + 2)
                nc.sync.dma_start(out=inm[:, :, :, :],
                                  in_=amain.ap()[:, :, z0:z0 + E + 2, :])
                int_t = io.tile([128, 5 * IT * WI], f16, tag="int")
                inT = int_t.rearrange("p (f z x) -> p f z x", f=5, z=IT)
                nc.sync.dma_start(out=inT[0:64, :, :, :],
                                  in_=atail.ap()[:, :, z0:z0 + IT, :])
                nc.sync.dma_start(out=inT[64:128, :, :, :],
                                  in_=atail.ap()[:, :, z0 + HC:z0 + HC + IT,
                                                 :])
                s = {"z0": z0, "inm": inm, "inT": inT}
                return s

            def stage_A(s):
                inm, inT = s["inm"], s["inT"]
                dm_t = wk.tile([128, 12 * E * W], f16, tag="dm")
                dm = dm_t.rearrange("p (s z x) -> p s z x", s=12, z=E)
                dt_t = wk.tile([128, 12 * ET * W], f16, tag="dt")
                dt = dt_t.rearrange("p (s z x) -> p s z x", s=12, z=ET)
                s["dm"], s["dt"] = dm, dt
                # z/x derivatives; D slots: 0..3 dz, 4..7 dy, 8..11 dx
                TT.tensor_sub(dm[:, 0:4, :, :],
                              inm[:, 0:4, 2:2 + E, 1:1 + W],
                              inm[:, 0:4, 0:E, 1:1 + W])
                TT.tensor_sub(dt[:, 0:4, :, :],
                              inT[:, 0:4, 2:2 + ET, 1:1 + W],
                              inT[:, 0:4, 0:ET, 1:1 + W])
                TT.tensor_sub(dm[:, 8:11:2, :, :],
                              inm[:, 0:3:2, 1:1 + E, 2:2 + W],
                              inm[:, 0:3:2, 1:1 + E, 0:W])
                TT.tensor_sub(dm[:, 9:12:2, 1:E - 1, :],
                              inm[:, 1:4:2, 2:E, 2:2 + W],
                              inm[:, 1:4:2, 2:E, 0:W])
                TT.tensor_sub(dt[:, 8:11:2, :, :],
                              inT[:, 0:3:2, 1:1 + ET, 2:2 + W],
                              inT[:, 0:3:2, 1:1 + ET, 0:W])
                TT.tensor_sub(dt[:, 9:12:2, 1:ET - 1, :],
                              inT[:, 1:4:2, 2:ET, 2:2 + W],
                              inT[:, 1:4:2, 2:ET, 0:W])

                # y derivatives (PE), drains into D slots 4..7
                def a_drain(sel, dst, pa):
                    src = pa.rearrange("p (a x) -> p a x", a=2)[:, :, 0:388] \
                        .rearrange("p a (f x) -> p a f x", f=2)
                    dstv = dst.rearrange("p (a f) x -> p a f x", a=2)
                    if sel:
                        TT.tensor_copy(out=dstv, in_=src)
                    else:
                        nc.scalar.copy(dstv, src)

                for lp in range(E):
                    pa = psA.tile([128, 1024], f32, tag="pa")
                    if lp < HC + 2:
                        kth, klt = 0, lp + 1
                        wkt = wk1m0b0[:, :]
                    else:
                        kth, klt = 64, lp - HC + 1
                        wkt = wk1m0b64[64:128, :]
                    for g in range(2):
                        pv = pa[:, 512 * g:512 * g + 388].rearrange(
                            "p (f x) -> p f x", f=2)
                        nc.tensor.matmul(pv[:, :, :], wk0m0[:, :],
                                         inm[:, 2 * g:2 * g + 2, lp + 1,
                                             1:1 + W],
                                         start=True, stop=False,
                                         skip_group_check=True)
                        nc.tensor.matmul(pv[:, :, :], wkt,
                                         inT[kth:kth + 64, 2 * g:2 * g + 2,
                                             klt, 1:1 + W],
                                         start=False, stop=True,
                                         skip_group_check=True)
                    a_drain(0, dm[:, 4:8, lp, :], pa)
                for lq in range(ET):
                    pa = psA.tile([128, 1024], f32, tag="pa")
                    for g in range(2):
                        for h, base in ((0, 0), (1, 64)):
                            lmain = lq + 1 + h * HC
                            pv = pa[base:base + 64,
                                    512 * g:512 * g + 388].rearrange(
                                "p (f x) -> p f x", f=2)
                            wkt = wk1m1b0[:, :] if h == 0 \
                                else wk1m1b64[64:128, :]
                            nc.tensor.matmul(pv[:, :, :], wk0m1[:, :],
                                             inm[:, 2 * g:2 * g + 2, lmain,
                                                 1:1 + W],
                                             start=True, stop=False,
                                             skip_group_check=True)
                            nc.tensor.matmul(pv[:, :, :], wkt,
                                             inT[base:base + 64,
                                                 2 * g:2 * g + 2, lq + 1,
                                                 1:1 + W],
                                             start=False, stop=True,
                                             skip_group_check=True)
                    a_drain(0, dt[:, 4:8, lq, :], pa)

            def stage_B1(s):
                # FL slots: [t00,t10,t20,ez|t01,t11,t21,ey|t02,t12,t22,ex]
                fm_t = wk.tile([128, 12 * E * W], f16, tag="fm")
                fm = fm_t.rearrange("p (s z x) -> p s z x", s=12, z=E)
                ft_t = wk.tile([128, 12 * ET * W], f16, tag="ft")
                ft = ft_t.rearrange("p (s z x) -> p s z x", s=12, z=ET)
                dvm_t = wk.tile([128, E * W], f16, tag="dv")
                dvm = dvm_t.rearrange("p (z x) -> p z x", z=E)
                dvt_t = wk.tile([128, ET * W], f16, tag="dvt")
                dvt = dvt_t.rearrange("p (z x) -> p z x", z=ET)
                s["fm"], s["ft"] = fm, ft
                s["dvm"], s["dvt"] = dvm, dvt
                for (d, f, dv, inp, ne) in (
                        (s["dm"], fm, dvm, s["inm"], E),
                        (s["dt"], ft, dvt, s["inT"], ET)):
                    m2 = inp[:, 4, 1:1 + ne, 1:1 + W]
                    m2b3 = m2.unsqueeze(1).broadcast_to((128, 3, ne, W))
                    # diag derivative slots doubled (u was staged halved;
                    # diag needs 2A' - (2/3)divu' = A'' - divu''/3)
                    TT.tensor_scalar_mul(d[:, 0:11:5, :, :],
                                         d[:, 0:11:5, :, :], 2.0)
                    TP.tensor_add(dv[:, :, :], d[:, 0, :, :], d[:, 5, :, :])
                    TP.tensor_add(dv[:, :, :], dv[:, :, :], d[:, 10, :, :])
                    TT.tensor_scalar_mul(dv[:, :, :], dv[:, :, :], 1.0 / 3.0)
                    # s00 needs full E (feeds Fz); s11/s22 interior only
                    lo, hi = 1, ne - 1
                    dvb2 = dv[:, lo:hi, :].unsqueeze(1).broadcast_to(
                        (128, 2, hi - lo, W))
                    TT.tensor_sub(f[:, 0, :, :], d[:, 0, :, :], dv[:, :, :])
                    TT.tensor_sub(f[:, 5:11:5, lo:hi, :],
                                  d[:, 5:11:5, lo:hi, :], dvb2)
                    # off-diag h: v->t01(4), w->t20(2), t->t21(6)
                    TP.tensor_add(f[:, 4, :, :], d[:, 1, :, :], d[:, 4, :, :])
                    TP.tensor_add(f[:, 2, :, :], d[:, 2, :, :], d[:, 8, :, :])
                    TP.tensor_add(f[:, 6, lo:hi, :], d[:, 6, lo:hi, :],
                                  d[:, 9, lo:hi, :])
                    # tau muls: diag (0,5,10); off-diag (2,4,6); e-seeds
                    TT.tensor_mul(f[:, 0, :, :], f[:, 0, :, :], m2)
                    TT.tensor_mul(f[:, 5:11:5, lo:hi, :],
                                  f[:, 5:11:5, lo:hi, :],
                                  m2[:, lo:hi, :].unsqueeze(1).broadcast_to(
                                      (128, 2, hi - lo, W)))
                    TP.tensor_mul(f[:, 2:5:2, :, :], f[:, 2:5:2, :, :],
                                  m2.unsqueeze(1).broadcast_to(
                                      (128, 2, ne, W)))
                    TP.tensor_mul(f[:, 6, lo:hi, :], f[:, 6, lo:hi, :],
                                  m2[:, lo:hi, :])
                    TT.tensor_mul(f[:, 3, :, :], d[:, 3, :, :], m2)
                    TT.tensor_mul(f[:, 7:12:4, lo:hi, :],
                                  d[:, 7:12:4, lo:hi, :],
                                  m2[:, lo:hi, :].unsqueeze(1).broadcast_to(
                                      (128, 2, hi - lo, W)))
                    # symmetric copies t10<-t01, (t02,t12)<-(t20,t21)
                    TP.tensor_copy(out=f[:, 1, :, :], in_=f[:, 4, :, :])
                    TP.tensor_copy(out=f[:, 8:10, lo:hi, :],
                                   in_=f[:, 2:7:4, lo:hi, :])

            def stage_B2(s):
                # e_j += sum_i tau_ij * u'_i; row-1 on Pool (parallel with
                # the DVE chain), rows 0/2 on DVE sharing dead slots 8:11
                for (d, f, dv, inp, ne) in (
                        (s["dm"], s["fm"], s["dvm"], s["inm"], E),
                        (s["dt"], s["ft"], s["dvt"], s["inT"], ET)):
                    # interior planes only for the full 3x3 product block
                    lo, hi = 1, ne - 1
                    p1 = d[:, 0:3, lo:hi, :]
                    nei = hi - lo
                    ub = [inp[:, i, 1 + lo:1 + hi, 1:1 + W].unsqueeze(1)
                          .broadcast_to((128, 3, nei, W)) for i in range(3)]
                    ev = f[:, 3:12:4, lo:hi, :]
                    TP.tensor_mul(p1[:, :, :, :],
                                  f[:, 1:12:4, lo:hi, :], ub[1])
                    TT.tensor_mul(d[:, 4:7, lo:hi, :],
                                  f[:, 2:12:4, lo:hi, :], ub[2])
                    TT.tensor_add(ev, ev, d[:, 4:7, lo:hi, :])
                    TT.tensor_mul(d[:, 8:11, lo:hi, :],
                                  f[:, 0:12:4, lo:hi, :], ub[0])
                    TT.tensor_add(ev, ev, d[:, 8:11, lo:hi, :])
                    TP.tensor_add(ev, ev, p1[:, :, :, :])
                    # edge planes (locals 0 and ne-1): only ez is consumed
                    # there (C's dz stream); add tau_i0*u'_i via dv scratch
                    zs = ne - 1
                    eze = f[:, 3, 0:ne:zs, :]
                    for i in range(3):
                        ue = inp[:, i, 1:ne + 1:zs, 1:1 + W]
                        TT.tensor_mul(dv[:, 0:2, :], f[:, i, 0:ne:zs, :], ue)
                        TT.tensor_add(eze, eze, dv[:, 0:2, :])

            def stage_C(s, g):
                fm, ft = s["fm"], s["ft"]
                if g == 0:
                    om_t = wk.tile([128, 4 * ZC * N], f16, tag="om")
                    s["om"] = om_t.rearrange("p (f z x) -> p f z x",
                                             f=4, z=ZC)
                    ot_t = wk.tile([128, 4 * HC * N], f16, tag="ot")
                    s["ot"] = ot_t.rearrange("p (f z x) -> p f z x",
                                             f=4, z=HC)
                om, ot = s["om"], s["ot"]
                q = 2 * g
                for k in range(ZC):
                    le = k + 1
                    pc = psC.tile([128, 512], f32, tag="pc")
                    pv = pc[:, 0:384].rearrange("p (f x) -> p f x", f=2)
                    if k < HC:
                        kth, klt = 0, k + 1
                        wkt = wk1m0b0[:, :]
                    else:
                        kth, klt = 64, k - HC + 1
                        wkt = wk1m0b64[64:128, :]
                    nc.tensor.matmul(pv[:, :, :], wi[:, :],
                                     fm[:, q:q + 2, le + 1, 1:193],
                                     start=True, stop=False,
                                     skip_group_check=True)
                    nc.tensor.matmul(pv[:, :, :], wim[:, :],
                                     fm[:, q:q + 2, le - 1, 1:193],
                                     start=False, stop=False,
                                     skip_group_check=True)
                    nc.tensor.matmul(pv[:, :, :], wi[:, :],
                                     fm[:, 8 + q:10 + q, le, 2:194],
                                     start=False, stop=False,
                                     skip_group_check=True)
                    nc.tensor.matmul(pv[:, :, :], wim[:, :],
                                     fm[:, 8 + q:10 + q, le, 0:192],
                                     start=False, stop=False,
                                     skip_group_check=True)
                    nc.tensor.matmul(pv[:, :, :], wk0m0[:, :],
                                     fm[:, 4 + q:6 + q, le, 1:193],
                                     start=False, stop=False,
                                     skip_group_check=True)
                    nc.tensor.matmul(pv[:, :, :], wkt,
                                     ft[kth:kth + 64, 4 + q:6 + q, klt,
                                        1:193],
                                     start=False, stop=True,
                                     skip_group_check=True)
                    nc.scalar.copy(om[:, q:q + 2, k, :], pv[:, :, :])
                for k in range(HC):
                    le = k + 1
                    pc = psC.tile([128, 512], f32, tag="pc")
                    pv = pc[:, 0:384].rearrange("p (f x) -> p f x", f=2)
                    nc.tensor.matmul(pv[:, :, :], wi[:, :],
                                     ft[:, q:q + 2, le + 1, 1:193],
                                     start=True, stop=False,
                                     skip_group_check=True)
                    nc.tensor.matmul(pv[:, :, :], wim[:, :],
                                     ft[:, q:q + 2, le - 1, 1:193],
                                     start=False, stop=False,
                                     skip_group_check=True)
                    nc.tensor.matmul(pv[:, :, :], wi[:, :],
                                     ft[:, 8 + q:10 + q, le, 2:194],
                                     start=False, stop=False,
                                     skip_group_check=True)
                    nc.tensor.matmul(pv[:, :, :], wim[:, :],
                                     ft[:, 8 + q:10 + q, le, 0:192],
                                     start=False, stop=False,
                                     skip_group_check=True)
                    for h, base in ((0, 0), (1, 64)):
                        pvh = pc[base:base + 64, 0:384].rearrange(
                            "p (f x) -> p f x", f=2)
                        wkt = wk1m1b0[:, :] if h == 0 \
                            else wk1m1b64[64:128, :]
                        nc.tensor.matmul(pvh[:, :, :], wk0m1[:, :],
                                         fm[:, 4 + q:6 + q,
                                            k + 1 + h * HC, 1:193],
                                         start=False, stop=False,
                                         skip_group_check=True)
                        nc.tensor.matmul(pvh[:, :, :], wkt,
                                         ft[base:base + 64, 4 + q:6 + q,
                                            le, 1:193],
                                         start=False, stop=(h == 1),
                                         skip_group_check=True)
                    nc.scalar.copy(ot[:, q:q + 2, k, :], pv[:, :, :])

            def stage_store(s):
                z0 = s["z0"]
                nc.sync.dma_start(out=omain.ap()[:, :, z0:z0 + ZC, :],
                                  in_=s["om"][:, :, :, :])
                nc.sync.dma_start(out=otail.ap()[:, :, z0:z0 + HC, :],
                                  in_=s["ot"][0:64, :, :, :])
                nc.sync.dma_start(out=otail.ap()[:, :, z0 + HC:z0 + ZC, :],
                                  in_=s["ot"][64:128, :, :, :])

            # software-pipelined emission: A(c+1) is emitted between B2(c)
            # and C-g1(c) so PE/ACT fill B2's shadow and B1(c+1) can start
            # during C-g1(c)
            st = stage_load(0)
            stage_A(st)
            for c in range(nchunks):
                nxt = stage_load(c + 1) if c + 1 < nchunks else None
                stage_B1(st)
                stage_C(st, 0)
                stage_B2(st)
                if nxt is not None:
                    stage_A(nxt)
                stage_C(st, 1)
                stage_store(st)
                st = nxt

    nc.compile()
    return nc


_NC_CACHE = None


def _get_nc():
    global _NC_CACHE
    if _NC_CACHE is None:
        _NC_CACHE = build_program()
    return _NC_CACHE


def make_wts() -> np.ndarray:
    dy = np.zeros((N, N), dtype=np.float32)
    for m in range(N):
        dy[m, (m + 1) % N] = 1.0
        dy[m, (m - 1) % N] = -1.0
    dyt = dy.T
    eye = np.eye(128, dtype=np.float32)
    w = np.zeros((192, 448), dtype=np.float32)
    w[:, 0:192] = dyt
    w[0:128, 192:320] = eye
    w[0:128, 320:448] = -eye
    return w.astype(np.float16)


def stage_fields(u, T):
    """Full-grid staged fields [5, N, N, WI] fp16 (x-halo'd, scaled)."""
    mu2 = (2.0 * MUSCALE) * np.power(T, 0.7, dtype=np.float32)
    tq = (CPR / 4.0) * T
    f = np.empty((5, N, N, N), dtype=np.float32)
    f[0:3] = 0.5 * u
    f[3] = tq
    f[4] = mu2
    fx = np.concatenate([f[..., -2:], f, f[..., :2]], axis=-1)
    return fx.astype(np.float16)


def shard_inputs(u, T, nz=NZ, ncores=NCORES):
    fx = stage_fields(u, T)          # [5, N, N, WI]
    wts = make_wts()
    in_maps = []
    for k in range(ncores):
        idx = np.arange(nz * k - 2, nz * k + nz + 2) % N
        blk = fx[:, idx, :, :]                     # [5, nz+4, N, WI]
        blk = np.ascontiguousarray(blk.transpose(2, 0, 1, 3))  # [y,5,z,x]
        in_maps.append({
            "amain": blk[0:128],
            "atail": np.ascontiguousarray(blk[128:192]),
            "wts": wts,
        })
    return in_maps


def kernel(u: np.ndarray, T: np.ndarray) -> np.ndarray:
    from concourse.bass_utils import run_bass_kernel_spmd

    u = np.asarray(u, dtype=np.float32)
    T = np.asarray(T, dtype=np.float32)
    nc = _get_nc()
    in_maps = shard_inputs(u, T)
    res = run_bass_kernel_spmd(nc, in_maps, list(range(NCORES)))

    out = np.zeros((5, N, N, N), dtype=np.float32)
    for k in range(NCORES):
        r = res.results[k]
        o = np.concatenate([np.asarray(r["omain"], dtype=np.float32),
                            np.asarray(r["otail"], dtype=np.float32)],
                           axis=0)                  # [192y, 4, nz, x]
        o = o.transpose(1, 2, 0, 3)                 # [4, nz, y, x]
        out[1:5, NZ * k:NZ * k + NZ] = o
    out[4] *= 2.0
    return out
